# revision 5
# baseline (speedup 1.0000x reference)
"""Multi-head scaled-dot-product attention on 8 Trainium2 NeuronCores.

Problem: x[4,2048,128], Wq/Wk/Wv[10,128,128] (torch Linear layout [e_out,d_in]),
Wo[128,1280], bo[128]  ->  out[4,2048,128]

Sharding: 8 cores = 4 batches x 2 head-groups (5 heads each). Each core
computes its batch's attention for its 5 heads plus the partial output
projection; the host sums the two half-head partials per batch, transposes,
and adds the bias.

Per-core layout strategy (all host-side pre-transposed, so no on-chip
transposes at all):
  xT   [d=128, n=2048]  = x[b].T
  wq/wk/wv [5, d, e]    = W*.transpose(0,2,1)   (so lhsT = W*T directly)
  wo   [5, e, dout]     = Wo.T.reshape(10,128,128)[head slice]
  QT_h [e, n]  = wqT_h.T @ xT          (matmul lhsT=wq, rhs=xT)
  KT_h [e, n]  = wkT_h.T @ xT
  V_h  [m, e]  = xT_chunk.T @ wvT_h    (natural layout, m on partitions)
  ST   [m-chunk, nb] = KT_slice.T @ QT_slice   (scores transposed: keys on
       partitions -> softmax denominator via ones-matmul, P^T is directly
       what the PV matmul needs as rhs)
  PT   = exp(ST / sqrt(D))             (ACT, no max-subtraction needed:
       scores are ~N(0,1), |S|<~7, exp is safe and exact in fp32)
  OT_h [e, nb] += V_chunk.T @ PT_chunk (accumulated over 16 m-chunks)
  den  [1, nb] += ones.T @ PT_chunk
  OTn  = OT * broadcast(1/den)         (K=1 ones matmul broadcasts recip)
  outT [dout, nb] += wo_h.T @ OTn      (accumulated over 5 heads)
"""

from contextlib import ExitStack

import numpy as np

import concourse.tile as tile
from concourse import bacc, mybir
from concourse.bass import ds, ts
from concourse.bass_utils import run_bass_kernel_spmd

B, N, D, H = 4, 2048, 128, 10
HL = H // 2  # heads per core
NCHUNK = N // 128  # 16 key chunks
NBLK = N // 512  # 4 query blocks
INV_SCALE = float(1.0 / (128.0**0.5 + 1e-8))
f32 = mybir.dt.float32

PROFILE = False
LAST_RESULTS = None

_built = None


def _emit(tc, xT, wq, wk, wv, wo, outT):
    nc = tc.nc
    Exp = mybir.ActivationFunctionType.Exp

    ctx = ExitStack()
    consts = ctx.enter_context(tc.tile_pool(name="consts", bufs=1))
    proj = ctx.enter_context(tc.tile_pool(name="proj", bufs=1))
    ps = ctx.enter_context(tc.tile_pool(name="ps", bufs=3, space="PSUM"))
    otps = ctx.enter_context(tc.tile_pool(name="otps", bufs=2, space="PSUM"))
    dnps = ctx.enter_context(tc.tile_pool(name="dnps", bufs=1, space="PSUM"))
    outps = ctx.enter_context(tc.tile_pool(name="outps", bufs=2, space="PSUM"))
    ptp = ctx.enter_context(tc.tile_pool(name="ptp", bufs=4))
    work = ctx.enter_context(tc.tile_pool(name="work", bufs=2))

    ones_col = consts.tile([128, 1], f32)
    nc.vector.memset(ones_col, 1.0)
    ones_row = consts.tile([1, 128], f32)
    nc.vector.memset(ones_row, 1.0)

    xT_sb = consts.tile([D, N], f32)
    for j in range(NBLK):
        nc.sync.dma_start(xT_sb[:, ts(j, 512)], xT[:, ts(j, 512)])
    wq_sb = consts.tile([D, HL * D], f32)
    wk_sb = consts.tile([D, HL * D], f32)
    wv_sb = consts.tile([D, HL * D], f32)
    wo_sb = consts.tile([D, HL * D], f32)
    for h in range(HL):
        nc.sync.dma_start(wq_sb[:, ts(h, D)], wq[h])
        nc.sync.dma_start(wk_sb[:, ts(h, D)], wk[h])
        nc.sync.dma_start(wv_sb[:, ts(h, D)], wv[h])
        nc.sync.dma_start(wo_sb[:, ts(h, D)], wo[h])

    qt = proj.tile([D, HL * N], f32)
    kt = proj.tile([D, HL * N], f32)
    vv = proj.tile([D, HL * N], f32)

    # ---- projections ----
    for h in range(HL):
        for j in range(NBLK):
            p = ps.tile([128, 512], f32, tag="st")
            nc.tensor.matmul(
                p[:], wq_sb[:, ts(h, D)], xT_sb[:, ts(j, 512)], start=True, stop=True
            )
            nc.vector.tensor_copy(qt[:, ds(h * N + j * 512, 512)], p[:])
        for j in range(NBLK):
            p = ps.tile([128, 512], f32, tag="st")
            nc.tensor.matmul(
                p[:], wk_sb[:, ts(h, D)], xT_sb[:, ts(j, 512)], start=True, stop=True
            )
            nc.vector.tensor_copy(kt[:, ds(h * N + j * 512, 512)], p[:])
        for cg in range(4):
            p = ps.tile([128, 512], f32, tag="st")
            for cc in range(4):
                c = cg * 4 + cc
                nc.tensor.matmul(
                    p[:, ts(cc, 128)],
                    xT_sb[:, ts(c, 128)],
                    wv_sb[:, ts(h, D)],
                    start=True,
                    stop=True,
                )
            nc.scalar.copy(vv[:, ds(h * N + cg * 512, 512)], p[:])

    # ---- attention (software-pipelined emission) ----
    # pending epilogue state from the previous (nb, h)
    pend = None  # dict with ot_ps, recip, outp, h, is_last_head

    def emit_bcast(st):
        bcp = ps.tile([128, 512], f32, tag="st")
        nc.tensor.matmul(bcp[:], ones_row[:], st["recip"][:], start=True, stop=True)
        bc = work.tile([128, 512], f32, tag="bc")
        nc.vector.tensor_copy(bc[:], bcp[:])
        st["bc"] = bc

    def emit_finish(st):
        otn = work.tile([128, 512], f32, tag="otn")
        nc.vector.tensor_mul(otn[:], st["ot_ps"][:], st["bc"][:])
        nc.tensor.matmul(
            st["outp"][:],
            wo_sb[:, ts(st["h"], D)],
            otn[:],
            start=(st["h"] == 0),
            stop=(st["h"] == HL - 1),
        )
        if st["h"] == HL - 1:
            osb = work.tile([128, 512], f32, tag="osb")
            nc.vector.tensor_copy(osb[:], st["outp"][:])
            nc.sync.dma_start(outT[:, ts(st["nb"], 512)], osb[:])

    for nb in range(NBLK):
        outp = outps.tile([128, 512], f32)
        for h in range(HL):
            ot_ps = otps.tile([128, 512], f32)
            dn_ps = dnps.tile([1, 512], f32)
            prev = None  # previous chunk's PT tile
            for c in range(NCHUNK):
                stp = ps.tile([128, 512], f32, tag="st")
                nc.tensor.matmul(
                    stp[:],
                    kt[:, ds(h * N + c * 128, 128)],
                    qt[:, ds(h * N + nb * 512, 512)],
                    start=True,
                    stop=True,
                )
                p = ptp.tile([128, 512], f32, tag="pt")
                nc.scalar.activation(p[:], stp[:], Exp, scale=INV_SCALE)
                if prev is not None:
                    pc, pp = prev
                    nc.tensor.matmul(
                        ot_ps[:],
                        vv[:, ds(h * N + pc * 128, 128)],
                        pp[:],
                        start=(pc == 0),
                        stop=(pc == NCHUNK - 1),
                    )
                    nc.tensor.matmul(
                        dn_ps[:],
                        ones_col[:],
                        pp[:],
                        start=(pc == 0),
                        stop=(pc == NCHUNK - 1),
                    )
                prev = (c, p)
                # interleave the previous head's epilogue into this head's
                # chunk stream so PE never waits on the DVE chain
                if pend is not None and c == 4:
                    emit_bcast(pend)
                if pend is not None and c == 10:
                    emit_finish(pend)
                    pend = None
            pc, pp = prev
            nc.tensor.matmul(
                ot_ps[:],
                vv[:, ds(h * N + pc * 128, 128)],
                pp[:],
                start=(pc == 0),
                stop=(pc == NCHUNK - 1),
            )
            nc.tensor.matmul(
                dn_ps[:],
                ones_col[:],
                pp[:],
                start=(pc == 0),
                stop=(pc == NCHUNK - 1),
            )
            recip = work.tile([1, 512], f32, tag="recip")
            nc.vector.reciprocal(recip[:], dn_ps[:])
            pend = {
                "ot_ps": ot_ps,
                "recip": recip,
                "outp": outp,
                "h": h,
                "nb": nb,
            }
    # flush the last epilogue
    emit_bcast(pend)
    emit_finish(pend)
    pend = None
    ctx.close()


def _build():
    nc = bacc.Bacc("TRN2", target_bir_lowering=False, debug=False)
    xT = nc.dram_tensor("xT", [D, N], f32, kind="ExternalInput").ap()
    wq = nc.dram_tensor("wq", [HL, D, D], f32, kind="ExternalInput").ap()
    wk = nc.dram_tensor("wk", [HL, D, D], f32, kind="ExternalInput").ap()
    wv = nc.dram_tensor("wv", [HL, D, D], f32, kind="ExternalInput").ap()
    wo = nc.dram_tensor("wo", [HL, D, D], f32, kind="ExternalInput").ap()
    outT = nc.dram_tensor("outT", [D, N], f32, kind="ExternalOutput").ap()
    with tile.TileContext(nc) as tc:
        _emit(tc, xT, wq, wk, wv, wo, outT)
    nc.compile()
    return nc


def kernel(x, Wq, Wk, Wv, Wo, bo):
    global _built, LAST_RESULTS
    x = np.asarray(x, dtype=np.float32)
    Wq = np.asarray(Wq, dtype=np.float32)
    Wk = np.asarray(Wk, dtype=np.float32)
    Wv = np.asarray(Wv, dtype=np.float32)
    Wo = np.asarray(Wo, dtype=np.float32)
    bo = np.asarray(bo, dtype=np.float32)

    if _built is None:
        _built = _build()
    nc = _built

    WqT = np.ascontiguousarray(Wq.transpose(0, 2, 1))
    WkT = np.ascontiguousarray(Wk.transpose(0, 2, 1))
    WvT = np.ascontiguousarray(Wv.transpose(0, 2, 1))
    WoT = np.ascontiguousarray(Wo.T.reshape(H, D, D))

    in_maps = []
    for c in range(8):
        b, g = divmod(c, 2)
        hsl = slice(g * HL, g * HL + HL)
        in_maps.append(
            {
                "xT": np.ascontiguousarray(x[b].T),
                "wq": WqT[hsl],
                "wk": WkT[hsl],
                "wv": WvT[hsl],
                "wo": WoT[hsl],
            }
        )

    res = run_bass_kernel_spmd(
        nc, in_maps, core_ids=list(range(8)), trace=PROFILE
    )
    LAST_RESULTS = res

    out = np.empty((B, N, D), dtype=np.float32)
    for b in range(B):
        oT = res.results[2 * b]["outT"] + res.results[2 * b + 1]["outT"]
        out[b] = oT.T
    out += bo
    return out
